# revision 48
# baseline (speedup 1.0000x reference)
"""DialogueGCN Trainium2 kernel — 8-core SPMD row-sharded implementation.

Decomposition (validated in numpy):
  attn = softmax(band(x@x.T)) has off-band entries equal to a per-row constant
  c_i = exp(-m_i)/Z_i.  Each relation adjacency adj_k = mask_k * attn splits into
    adj_k @ s = [A_k^ext @ s_ext]   (per-96-row-block: c_i*mask within own block
                                     + band corrections over +-10 cols)
    + c_i * (E_rows @ H_k)          (cross-block per-speaker-class prefix/suffix
                                     sums of s, via a small AllGather of per-block
                                     class sums G)
  Weight basis repack W' = [W1-W3, W2-W4, W3, W4] makes the diff-speaker masks
  implicit: sum_k A_k s_k = (w1*sm)s'1 + (w2*sm)s'2 + w1 s'3 + w2 s'4, and the
  cross term needs only E-weighted sums for chunks 1/2 plus plain block sums
  (a ones row in the 9-row class matrix) for chunks 3/4.
  Mini-blocks (10 halo rows each side) replicate neighbour-core h1 rows locally
  so layer 2 needs no halo exchange.
"""
import os
import sys

for _p in ("/opt/trn_rl_repo", "/root/.axon_site/_ro/trn_rl_repo"):
    if os.path.isdir(_p) and _p not in sys.path:
        sys.path.insert(0, _p)

import numpy as np
import ml_dtypes

import concourse.bass as bass
import concourse.mybir as mybir
import concourse.tile as tile
from concourse import masks
from concourse.bass_utils import run_bass_kernel_spmd

N, D, WIN, NSPK, NEMO = 6144, 128, 10, 8, 7
CORES, R, B, NBL = 8, 768, 96, 8
EXT = B + 2 * WIN          # 116
HALO = B + WIN             # 106
XR = R + 2 * HALO          # 980
NBG = CORES * NBL          # 64
NCLS = NSPK + 1            # 8 speaker classes + block-sum row
F32 = mybir.dt.float32
BF16 = mybir.dt.bfloat16
AOT = mybir.AluOpType
ACTF = mybir.ActivationFunctionType

# block geometry: (t, ostart, P, estart, mini_col)  in local l coords
FULL_TS = [(t, HALO + B * t, B, B + B * t, None) for t in range(NBL)]
MINI_TS = [(8, B, WIN, 0, 0), (9, HALO + R, WIN, XR - EXT, 1)]

# ---------------- blob layouts (shared host/device) ----------------
# bf16 blob items: (name, shape); partition dim first, free dims flattened.
BLOB_B = [
    ("w41", (D, 4 * D)), ("w42", (D, 4 * D)),
    ("wag1", (D, D)), ("wag2", (D, D)),
    ("we1a", (D, D)), ("we1b", (D, D)), ("we2", (D, NEMO)),
    ("wsa", (D, NEMO)), ("wsb", (D, NEMO)),
]
# short-partition bf16 items, packed at height 116 (exact-size upload)
BLOB_S = [
    ("eO", (EXT, NBL, NCLS)),
    ("pred3", (B, EXT)), ("suc3", (B, EXT)), ("diagm3", (B, EXT)),
    ("predib", (B, EXT)), ("sucib", (B, EXT)),
    ("cm_pred", (WIN, 2, EXT)), ("cm_suc", (WIN, 2, EXT)),
    ("cm_diagm", (WIN, 2, EXT)), ("cm_predib", (WIN, 2, EXT)),
    ("cm_sucib", (WIN, 2, EXT)),
    ("triS", (NBG, 10)), ("triP", (NBG, 10)),
    ("e9Tm", (NCLS, 2 * WIN)),
]
BLOB_F = [
    ("band", (B, EXT)), ("cm_band", (WIN, 2, EXT)),
    ("vmask", (WIN, 2)), ("be1", (D, 1)), ("be2", (NEMO, 1)), ("bs", (NEMO, 1)),
]


def _blob_layout(spec):
    offs, col = {}, 0
    for name, shape in spec:
        w = int(np.prod(shape[1:]))
        offs[name] = (col, shape)
        col += w
    return offs, col


OFF_B, COLS_B = _blob_layout(BLOB_B)
OFF_S, COLS_S = _blob_layout(BLOB_S)
OFF_F, COLS_F = _blob_layout(BLOB_F)


def build_program():
    nc = bass.Bass()
    dp = nc.declare_dram_parameter

    xT_d = dp("xT", [D, XR], F32, isOutput=False)
    xTb_d = dp("xTb", [D, XR], BF16, isOutput=False)
    blobB_d = dp("blobB", [D, COLS_B], BF16, isOutput=False)
    blobS_d = dp("blobS", [EXT, COLS_S], BF16, isOutput=False)
    blobF_d = dp("blobF", [D, COLS_F], F32, isOutput=False)
    eT_d = dp("eT", [NSPK, XR], BF16, isOutput=False)
    e9T_d = dp("e9T", [NCLS, R], BF16, isOutput=False)
    emo_d = dp("emoT", [NEMO, R], F32, isOutput=True)
    sen_d = dp("senT", [NEMO, R], F32, isOutput=True)

    # agi stored block-major so the AllGather concat yields flat [NBG, NCLS, 2D]
    # rows 0-7: class sums of s'1/s'2 chunks; row 8: block sums of s'3/s'4.
    ag_in = [nc.dram_tensor(f"ag{L}_in", [NBL, NCLS, 2 * D], BF16) for L in (1, 2)]
    ag_out = [
        nc.dram_tensor(f"ag{L}_out", [NBG, NCLS, 2 * D], BF16,
                       addr_space="Shared")
        for L in (1, 2)
    ]

    with tile.TileContext(nc) as tc:
        with tc.tile_pool(name="persist", bufs=1) as pp:
            # ---- load inputs / constants (few, batched DMAs) ----
            # order matters: the G-path (w4, xTb) gates the AllGather trigger
            blobB = pp.tile([D, COLS_B], BF16)
            nc.sync.dma_start(out=blobB[:], in_=blobB_d[:])
            xTb = pp.tile([D, XR], BF16)
            nc.sync.dma_start(out=xTb[:], in_=xTb_d[:])
            xT = pp.tile([D, XR], F32)
            nc.sync.dma_start(out=xT[:], in_=xT_d[:])
            blobS = pp.tile([EXT, COLS_S], BF16)
            nc.sync.dma_start(out=blobS[:], in_=blobS_d[:])
            blobF = pp.tile([D, COLS_F], F32)
            nc.sync.dma_start(out=blobF[:], in_=blobF_d[:])
            eTt = pp.tile([NSPK, XR], BF16)
            nc.scalar.dma_start(out=eTt[:], in_=eT_d[:])
            e9Tt = pp.tile([NCLS, R], BF16)
            nc.scalar.dma_start(out=e9Tt[:], in_=e9T_d[:])

            def _view(tile_, offs, name):
                off, shape = offs[name]
                w = int(np.prod(shape[1:]))
                v = tile_[0:shape[0], off:off + w]
                if len(shape) == 3:
                    v = v.rearrange("p (a b) -> p a b", b=shape[2])
                return v

            def cb(name):
                return _view(blobB, OFF_B, name)

            def cs(name):
                return _view(blobS, OFF_S, name)

            def cf(name):
                return _view(blobF, OFF_F, name)

            idf = pp.tile([128, 128], F32)
            masks.make_identity(nc, idf[:])
            idb = pp.tile([128, 128], BF16)
            masks.make_identity(nc, idb[:])

            # ---- persistent state tiles ----
            h1T = pp.tile([D, R + 2 * WIN], BF16)       # col = l - 96
            h2T = pp.tile([D, R], BF16)
            cB = pp.tile([B, NBL], F32)
            dB = pp.tile([B, NBL], F32)
            cM = pp.tile([WIN, 2], F32)
            dM = pp.tile([WIN, 2], F32)
            AT = {}
            for t, _, P, _, _ in FULL_TS + MINI_TS:
                for k in range(4):
                    AT[(k, t)] = pp.tile([EXT, P], BF16, name=f"AT{k}_{t}")
            accM = {}
            accA = {}
            for t, _, P, _, _ in FULL_TS + MINI_TS:
                accM[(t, 1)] = pp.tile([P, D], F32, name=f"accM1_{t}")
                accA[(t, 1)] = pp.tile([P, D], F32, name=f"accA1_{t}")
                if t < NBL:
                    accM[(t, 2)] = pp.tile([P, D], F32, name=f"accM2_{t}")
                    accA[(t, 2)] = pp.tile([P, D], F32, name=f"accA2_{t}")

            # =============== layer part 1: s, G, AllGather ===============
            def layer_part1(L, hT, hoff, w4, agi, ago, sp, psp, psg, gall,
                            ts_list):
                s_tiles = {}
                eO = cs("eO")
                for i, (t, ostart, P, estart, _) in enumerate(ts_list):
                    pss = psp.tile([EXT, 4 * D], F32, name=f"pss{L}", tag="pss")
                    nc.tensor.matmul(
                        pss[:], hT[:, estart - hoff:estart - hoff + EXT],
                        w4, start=True, stop=True)
                    sAll = sp.tile([EXT, 4 * D], BF16, name=f"sAll{L}_{t}")
                    # split the copy across engines: it is on the G critical path
                    nc.vector.tensor_copy(sAll[:, 0:2 * D], pss[:, 0:2 * D])
                    nc.scalar.copy(sAll[:, 2 * D:4 * D], pss[:, 2 * D:4 * D])
                    s_tiles[t] = sAll
                    if t < NBL:
                        # classes 0-7 need chunks 1-2; the block-sum row needs
                        # chunks 3-4 (ones-row trick)
                        ps2 = psg.tile([NSPK, 2 * D], F32, name=f"psg{L}",
                                       tag="psg")
                        nc.tensor.matmul(
                            ps2[:], eO[:, t, 0:NSPK], sAll[:, 0:2 * D],
                            start=True, stop=True)
                        ps2b = psg.tile([1, 2 * D], F32, name=f"psgb{L}",
                                        tag="psgb")
                        nc.tensor.matmul(
                            ps2b[:], eO[:, t, NSPK:NCLS],
                            sAll[:, 2 * D:4 * D], start=True, stop=True)
                        gallC, gallT = gall
                        (nc.vector.tensor_copy if i % 2 else nc.scalar.copy)(
                            gallC[:, t, :], ps2[:])
                        (nc.scalar.copy if i % 2 else nc.vector.tensor_copy)(
                            gallT[:, t, :], ps2b[:])
                with tc.high_priority():
                    gallC, gallT = gall
                    nc.sync.dma_start(
                        out=agi[:, 0:NSPK, :].rearrange("b n d -> n b d"),
                        in_=gallC[:])
                    nc.sync.dma_start(
                        out=agi[:, NSPK:NCLS, :].rearrange("b n d -> n b d"),
                        in_=gallT[:])
                    nc.gpsimd.collective_compute(
                        "AllGather", AOT.bypass,
                        replica_groups=[list(range(CORES))],
                        ins=[agi[:]], outs=[ago[:]],
                    )
                return s_tiles

            def layer_aggr(L, hT, hoff, wag, ts_list, psagg):
                for t, ostart, P, estart, _ in ts_list:
                    pag = psagg.tile([B, D], F32, name=f"pag{L}", tag="pag")
                    nc.tensor.matmul(
                        pag[:P, :], hT[:, ostart - hoff:ostart - hoff + P],
                        wag, start=True, stop=True)
                    nc.vector.tensor_copy(accA[(t, L)][:], pag[:P, :])

            # =============== attention math (layer independent) ===============
            def a_build(ab, ps_tr, blocks, PP, nb, cd, sb, sm, c_out, d_out,
                        tag):
                """sb/sm: [PP, nb, EXT] banded scores / same masks (pre-filled).
                cd: dict of mask APs — pred3/suc3/diagm3 broadcastable
                [PP, nb, EXT]; predib/sucib per-j [PP, EXT] fns."""
                sh3 = [PP, nb, EXT]
                mB = ab.tile([PP, nb], F32, name=f"mB{tag}")       # holds -m
                nc.vector.tensor_reduce(
                    mB[:], sb[:], axis=mybir.AxisListType.X, op=AOT.max,
                    negate=True)
                exv = ab.tile(sh3, BF16, name=f"exv{tag}")
                sumB = ab.tile([PP, nb], F32, name=f"sumB{tag}")
                for j in range(nb):
                    nc.scalar.activation(
                        exv[:, j, :], sb[:, j, :], ACTF.Exp,
                        bias=mB[:, j:j + 1], accum_out=sumB[:, j:j + 1])
                enB = ab.tile([PP, nb], F32, name=f"enB{tag}")
                nc.scalar.activation(enB[:], mB[:], ACTF.Exp)
                ZB = ab.tile([PP, nb], F32, name=f"ZB{tag}")
                nc.vector.scalar_tensor_tensor(
                    ZB[:], enB[:], float(N - EXT), sumB[:], AOT.mult, AOT.add)
                rZ = ab.tile([PP, nb], F32, name=f"rZ{tag}")
                nc.vector.reciprocal(rZ[:], ZB[:])
                nc.vector.tensor_tensor(c_out, enB[:], rZ[:], AOT.mult)
                dg = ab.tile(sh3, BF16, name=f"dg{tag}")
                nc.vector.tensor_tensor(dg[:], exv[:], cd["diagm3"], AOT.mult)
                d0 = ab.tile([PP, nb], F32, name=f"d0{tag}")
                nc.vector.tensor_reduce(
                    d0[:], dg[:], axis=mybir.AxisListType.X, op=AOT.add)
                nc.vector.tensor_tensor(d_out, d0[:], rZ[:], AOT.mult)
                negc = ab.tile([PP, nb], F32, name=f"negc{tag}")
                nc.vector.tensor_scalar_mul(negc[:], c_out, -1.0)
                u = ab.tile(sh3, BF16, name=f"u{tag}")
                for j in range(nb):
                    # u = (exv - enB)*rZ = exv*rZ - c, on the scalar engine
                    nc.scalar.activation(
                        u[:, j, :], exv[:, j, :], ACTF.Identity,
                        bias=negc[:, j:j + 1], scale=rZ[:, j:j + 1])
                up = ab.tile(sh3, BF16, name=f"up{tag}")
                nc.vector.tensor_tensor(up[:], u[:], cd["pred3"], AOT.mult)
                un = ab.tile(sh3, BF16, name=f"un{tag}")
                nc.vector.tensor_tensor(un[:], u[:], cd["suc3"], AOT.mult)
                w1 = ab.tile(sh3, BF16, name=f"w1{tag}")
                w2 = ab.tile(sh3, BF16, name=f"w2{tag}")
                for j in range(nb):
                    nc.vector.scalar_tensor_tensor(
                        w1[:, j, :], cd["predib"](j), c_out[:, j:j + 1],
                        up[:, j, :], AOT.mult, AOT.add)
                    nc.vector.scalar_tensor_tensor(
                        w2[:, j, :], cd["sucib"](j), c_out[:, j:j + 1],
                        un[:, j, :], AOT.mult, AOT.add)
                A0 = ab.tile(sh3, BF16, name=f"A0{tag}")
                A1 = ab.tile(sh3, BF16, name=f"A1{tag}")
                nc.vector.tensor_tensor(A0[:], w1[:], sm[:], AOT.mult)
                nc.vector.tensor_tensor(A1[:], w2[:], sm[:], AOT.mult)
                srcs = (A0, A1, w1, w2)
                for j, (t, ostart, P, estart, _) in enumerate(blocks):
                    for k in range(4):
                        pst = ps_tr.tile([EXT, PP], BF16, name="pst", tag="pst")
                        nc.tensor.matmul(
                            pst[:, :P], srcs[k][:P, j, :], idb[:P, :P],
                            is_transpose=True, start=True, stop=True)
                        nc.any.tensor_copy(AT[(k, t)][:], pst[:, :P])

            def part2_order(ts_list):
                if len(ts_list) <= NBL:
                    return ts_list
                by_t = {t[0]: t for t in ts_list}
                order = [8, 0, 1, 2, 3, 4, 5, 6, 7, 9]
                return [by_t[t] for t in order]

            # =============== layer part 2: A-matmuls, H, cross, combine =======
            def layer_part2(L, ago, s_tiles, ts_list):
                ts_list = part2_order(ts_list)
                NT = len(ts_list)
                with tc.tile_pool(name=f"psA{L}", bufs=2, space="PSUM") as psa:
                    for t, ostart, P, estart, mcol in ts_list:
                        pm = psa.tile([P, D], F32, name=f"pm{L}", tag="pm")
                        for k in range(4):
                            nc.tensor.matmul(
                                pm[:], AT[(k, t)][:, :P],
                                s_tiles[t][:, k * D:(k + 1) * D],
                                start=(k == 0), stop=(k == 3))
                        dsl = (dB[:, t:t + 1] if t < NBL
                               else dM[:, mcol:mcol + 1])
                        # accM = aggr*d + sum_k A_k @ s_k
                        nc.vector.scalar_tensor_tensor(
                            accM[(t, L)][:], accA[(t, L)][:], dsl, pm[:],
                            AOT.mult, AOT.add)
                triS, triP = cs("triS"), cs("triP")
                with tc.tile_pool(name=f"hL{L}", bufs=1) as hp:
                    gf = hp.tile([NBG, NCLS, 2, D], BF16, name=f"gf{L}")
                    nc.sync.dma_start(
                        out=gf[:],
                        in_=ago[:].rearrange("g n (r e) -> g n r e", e=D))
                    hm9 = hp.tile([NT, NCLS, D], BF16, name=f"hm9{L}")
                    with tc.tile_pool(name=f"psH{L}", bufs=2,
                                      space="PSUM") as psh:
                        for c0 in (0, 4):
                            ph = psh.tile([NT, 4 * D], F32, name=f"ph{L}",
                                          tag="ph")
                            nc.tensor.matmul(
                                ph[:], triS[:, :NT], gf[:, c0:c0 + 4, 0, :],
                                start=True, stop=False)
                            nc.tensor.matmul(
                                ph[:], triP[:, :NT], gf[:, c0:c0 + 4, 1, :],
                                start=False, stop=True)
                            (nc.vector.tensor_copy if c0 else nc.scalar.copy)(
                                hm9[:, c0:c0 + 4, :], ph[:])
                        pt23 = psh.tile([NT, D], F32, name=f"pt23{L}",
                                        tag="ph")
                        nc.tensor.matmul(
                            pt23[:], triS[:, :NT], gf[:, NSPK, 0, :],
                            start=True, stop=False)
                        nc.tensor.matmul(
                            pt23[:], triP[:, :NT], gf[:, NSPK, 1, :],
                            start=False, stop=True)
                        nc.scalar.copy(hm9[:, NSPK, :], pt23[:])
                    e9T, e9Tm = e9Tt[:], cs("e9Tm")
                    vmask = cf("vmask")
                    with tc.tile_pool(name=f"xb{L}", bufs=1) as xb, \
                         tc.tile_pool(name=f"psX{L}", bufs=2,
                                      space="PSUM") as psx:
                        hm4s = {}
                        for j, (t, ostart, P, estart, mcol) in enumerate(
                                ts_list):
                            hm4 = xb.tile([NCLS, D], BF16, name=f"hm4{L}_{t}")
                            eng = (nc.scalar, nc.gpsimd, nc.sync)[j % 3]
                            eng.dma_start(out=hm4[:], in_=hm9[t:t + 1, :, :])
                            hm4s[t] = hm4
                        for t, ostart, P, estart, mcol in ts_list:
                            pc = psx.tile([P, D], F32, name=f"pc{L}", tag="pc",
                                          bufs=3)
                            if t < NBL:
                                e9sl = e9T[:, B * t:B * t + P]
                            else:
                                e9sl = e9Tm[:, mcol * WIN:(mcol + 1) * WIN]
                            nc.tensor.matmul(
                                pc[:], e9sl, hm4s[t][:], start=True, stop=True)
                            csl = (cB[:, t:t + 1] if t < NBL
                                   else cM[:, mcol:mcol + 1])
                            hrow = xb.tile([P, D], BF16, name=f"hrow{L}",
                                           tag="hrow", bufs=4)
                            nc.vector.scalar_tensor_tensor(
                                hrow[:], pc[:], csl, accM[(t, L)][:],
                                AOT.mult, AOT.add)
                            if t >= NBL:
                                nc.vector.tensor_scalar_mul(
                                    hrow[:], hrow[:], vmask[:, mcol:mcol + 1])
                            ptr = psx.tile([D, P], BF16, name=f"ptr{L}",
                                           tag="ptr", bufs=3)
                            nc.tensor.matmul(
                                ptr[:], hrow[:], idb[:P, :P],
                                is_transpose=True, start=True, stop=True)
                            if L == 1:
                                off = {8: 0, 9: R + WIN}.get(t, WIN + B * t)
                                nc.scalar.activation(
                                    h1T[:, off:off + P], ptr[:], ACTF.Relu)
                            else:
                                nc.scalar.activation(
                                    h2T[:, B * t:B * t + P], ptr[:], ACTF.Relu)

            # =============== head: two 384-wide chunks over h2T ===============
            def head():
                CH = 4 * B
                we1a, we1b, we2 = cb("we1a"), cb("we1b"), cb("we2")
                wsa, wsb = cb("wsa"), cb("wsb")
                be1, be2, bs = cf("be1"), cf("be2"), cf("bs")
                with tc.tile_pool(name="hd", bufs=2) as hd, \
                     tc.tile_pool(name="psE", bufs=2, space="PSUM") as pse:
                    for c0 in (0, CH):
                        h2c = h2T[:, c0:c0 + CH]
                        xc_ = xTb[:, HALO + c0:HALO + c0 + CH]
                        pe1 = pse.tile([D, CH], F32, name="pe1", tag="pe1")
                        nc.tensor.matmul(pe1[:], we1b, xc_,
                                         start=True, stop=False)
                        nc.tensor.matmul(pe1[:], we1a, h2c,
                                         start=False, stop=True)
                        e1b = hd.tile([D, CH], BF16, name="e1b", tag="e1b")
                        nc.scalar.activation(e1b[:], pe1[:], ACTF.Relu,
                                             bias=be1)
                        pe2 = pse.tile([NEMO, CH], F32, name="pe2", tag="pe2")
                        nc.tensor.matmul(pe2[:], we2, e1b[:],
                                         start=True, stop=True)
                        em1 = hd.tile([NEMO, CH], F32, name="em1", tag="em1")
                        nc.vector.tensor_scalar_add(em1[:], pe2[:], be2)
                        nc.sync.dma_start(out=emo_d[:, c0:c0 + CH], in_=em1[:])
                        ps2 = pse.tile([NEMO, CH], F32, name="ps2", tag="pe2")
                        nc.tensor.matmul(ps2[:], wsb, xc_,
                                         start=True, stop=False)
                        nc.tensor.matmul(ps2[:], wsa, h2c,
                                         start=False, stop=True)
                        sn1 = hd.tile([NEMO, CH], F32, name="sn1", tag="em1")
                        nc.vector.tensor_scalar_add(sn1[:], ps2[:], bs)
                        nc.sync.dma_start(out=sen_d[:, c0:c0 + CH], in_=sn1[:])

            # =============== orchestrate ===============
            L1_TS = FULL_TS + MINI_TS
            with tc.tile_pool(name="abuild", bufs=1) as ab:
                sbF = ab.tile([B, NBL, EXT], F32, name="sbF")
                smF = ab.tile([B, NBL, EXT], BF16, name="smF")
                sbM = ab.tile([WIN, 2, EXT], F32, name="sbM")
                smM = ab.tile([WIN, 2, EXT], BF16, name="smM")
                with tc.tile_pool(name="sL1", bufs=1) as sp1, \
                     tc.tile_pool(name="gL1", bufs=1) as gp1:
                    gall1 = (gp1.tile([NSPK, NBL, 2 * D], BF16, name="gall1C"),
                             gp1.tile([1, NBL, 2 * D], BF16, name="gall1T"))
                    eT = eTt[:]
                    band, cm_band = cf("band"), cf("cm_band")
                    # warm the PE HAM clock (3.4us of sustained activity
                    # flips it 1.2->2.4GHz), then same-mask matmuls (only
                    # need the small eT DMA), before the G critical path.
                    with tc.tile_pool(name="ps_wm", bufs=2, space="PSUM") as ps_wm:
                        for w in range(16):
                            pw = ps_wm.tile([128, 128], F32, name="pw",
                                            tag="pw")
                            nc.tensor.matmul(pw[:], idb[:], idb[:],
                                             start=True, stop=True)
                        for t, ostart, P, estart, _ in L1_TS:
                            j = t if t < NBL else t - NBL
                            sm_t = smF if t < NBL else smM
                            pssm = ps_wm.tile([B, EXT], F32, name="pssm",
                                              tag="pw")
                            nc.tensor.matmul(
                                pssm[:P, :], eT[:, ostart:ostart + P],
                                eT[:, estart:estart + EXT], start=True,
                                stop=True)
                            (nc.vector.tensor_copy if j % 2 else nc.scalar.copy)(
                                sm_t[:P, j, :], pssm[:P, :])
                    with tc.tile_pool(name="psL1", bufs=3, space="PSUM") as psp1, \
                         tc.tile_pool(name="psG1", bufs=2, space="PSUM") as psg1:
                        s1 = layer_part1(1, xTb[:], 0, cb("w41"), ag_in[0],
                                         ag_out[0], sp1, psp1, psg1, gall1,
                                         L1_TS)
                    # banded scores for all blocks
                    with tc.tile_pool(name="ps_sc", bufs=2, space="PSUM") as ps_sc:
                        for t, ostart, P, estart, _ in L1_TS:
                            j = t if t < NBL else t - NBL
                            sb_t = sbF if t < NBL else sbM
                            bandap = (band[:P] if t < NBL
                                      else cm_band[:, j, :])
                            pssc = ps_sc.tile([B, EXT], F32, name="pssc",
                                              tag="pssc")
                            nc.tensor.matmul(
                                pssc[:P, :], xT[:, ostart:ostart + P],
                                xT[:, estart:estart + EXT], start=True,
                                stop=True)
                            nc.vector.tensor_tensor(
                                sb_t[:P, j, :], pssc[:P, :], bandap, AOT.mult)
                    with tc.tile_pool(name="psAg", bufs=2, space="PSUM") as psagg_:
                        layer_aggr(1, xTb[:], 0, cb("wag1"), L1_TS, psagg_)
                    with tc.tile_pool(name="ps_tr", bufs=2, space="PSUM") as ps_tr:
                        def bcast8(ap):
                            return ap.rearrange(
                                "p (a e) -> p a e", a=1).broadcast_to(
                                    [B, NBL, EXT])

                        cd_full = {
                            "predib": lambda j: cs("predib"),
                            "sucib": lambda j: cs("sucib"),
                            "pred3": bcast8(cs("pred3")),
                            "suc3": bcast8(cs("suc3")),
                            "diagm3": bcast8(cs("diagm3")),
                        }
                        a_build(ab, ps_tr, FULL_TS, B, NBL, cd_full,
                                sbF[:], smF[:], cB[:], dB[:], "F")
                        cd_mini = {
                            "predib": lambda j: cs("cm_predib")[:, j, :],
                            "sucib": lambda j: cs("cm_sucib")[:, j, :],
                            "pred3": cs("cm_pred")[:],
                            "suc3": cs("cm_suc")[:],
                            "diagm3": cs("cm_diagm")[:],
                        }
                        a_build(ab, ps_tr, MINI_TS, WIN, 2, cd_mini,
                                sbM[:], smM[:], cM[:], dM[:], "M")
                    layer_part2(1, ag_out[0], s1, L1_TS)
            with tc.tile_pool(name="sL2", bufs=1) as sp2, \
                 tc.tile_pool(name="gL2", bufs=1) as gp2:
                gall2 = (gp2.tile([NSPK, NBL, 2 * D], BF16, name="gall2C"),
                         gp2.tile([1, NBL, 2 * D], BF16, name="gall2T"))
                with tc.tile_pool(name="psL2", bufs=3, space="PSUM") as psp2, \
                     tc.tile_pool(name="psG2", bufs=2, space="PSUM") as psg2:
                    s2 = layer_part1(2, h1T[:], B, cb("w42"), ag_in[1],
                                     ag_out[1], sp2, psp2, psg2, gall2,
                                     FULL_TS)
                with tc.tile_pool(name="psAg2", bufs=2, space="PSUM") as psagg_:
                    layer_aggr(2, h1T[:], B, cb("wag2"), FULL_TS, psagg_)
                layer_part2(2, ag_out[1], s2, FULL_TS)
            head()

    split_multi_waits(nc)
    return nc


def split_multi_waits(nc, max_waits=1):
    """walrus only supports one sync-wait per instruction; hoist extras onto
    single-wait NoOps on the same engine queue."""
    n_fixed = 0
    for f in nc.m.functions:
        for bb in f.blocks:
            insts = list(bb.instructions)
            new_insts = []
            changed = False
            for ins in insts:
                si = getattr(ins, "sync_info", None)
                if si is not None and len(si.on_wait) > max_waits:
                    extra = list(si.on_wait)[:-max_waits]
                    keep = list(si.on_wait)[-max_waits:]
                    for j, w in enumerate(extra):
                        nop = mybir.InstNoOp(
                            name=f"wh{j}-{ins.name}", ins=[], outs=[],
                            engine=ins.engine,
                            sync_info=mybir.SyncInfo(on_wait=[w], on_update=[]),
                        )
                        new_insts.append(nop)
                    ins.sync_info = mybir.SyncInfo(
                        on_wait=keep, on_update=list(si.on_update))
                    changed = True
                    n_fixed += 1
                new_insts.append(ins)
            if changed:
                bb.instructions = new_insts
    return n_fixed


# ---------------- host-side input prep ----------------

def _consts_np():
    ii = np.arange(B)[:, None]
    cc = np.arange(EXT)[None, :]
    c = {}
    c["band"] = ((cc - ii >= 0) & (cc - ii <= 2 * WIN)).astype(np.float32)
    c["pred"] = ((cc - ii >= WIN) & (cc - ii <= 2 * WIN)).astype(np.float32)
    c["suc"] = ((cc - ii >= 0) & (cc - ii <= WIN - 1)).astype(np.float32)
    c["predib"] = ((cc >= ii + WIN) & (cc >= WIN) & (cc < WIN + B)).astype(np.float32)
    c["sucib"] = ((cc < ii + WIN) & (cc >= WIN) & (cc < WIN + B)).astype(np.float32)
    c["diagm"] = (cc == ii + WIN).astype(np.float32)
    cm = {}
    for n, v in c.items():
        cm[n] = np.stack([v[B - WIN:B], v[0:WIN]], axis=1).copy()  # [WIN, 2, EXT]
    return c, cm


def _pack_blob(items, spec, total, np_dtype, height):
    blob = np.zeros((height, total), np_dtype)
    offs, _ = _blob_layout(spec)
    for name, (off, shape) in offs.items():
        v = np.asarray(items[name], np_dtype).reshape(shape[0], -1)
        blob[0:shape[0], off:off + v.shape[1]] = v
    return blob


def make_in_maps(inputs):
    x = np.asarray(inputs["x"], np.float32)
    spk = np.asarray(inputs["speakers"])
    E = np.zeros((N, NSPK), np.float32)
    E[np.arange(N), spk] = 1.0
    xg = np.zeros((N + 2 * HALO, D), np.float32)
    xg[HALO:HALO + N] = x
    Eg = np.zeros((N + 2 * HALO, NSPK), np.float32)
    Eg[HALO:HALO + N] = E

    # W' basis: [W1-W3, W2-W4, W3, W4]
    def wrepack(Wp, Ws, Wm, Wd):
        return np.concatenate(
            [np.asarray(Wp, np.float32) - np.asarray(Wm, np.float32),
             np.asarray(Ws, np.float32) - np.asarray(Wd, np.float32),
             np.asarray(Wm, np.float32), np.asarray(Wd, np.float32)], axis=1)

    w41 = wrepack(inputs["W_pred1"], inputs["W_suc1"],
                  inputs["W_same1"], inputs["W_diff1"])
    w42 = wrepack(inputs["W_pred2"], inputs["W_suc2"],
                  inputs["W_same2"], inputs["W_diff2"])
    cfull, cmini = _consts_np()

    sharedB = {
        "w41": w41, "w42": w42,
        "wag1": inputs["w_aggr_1"], "wag2": inputs["w_aggr_2"],
        "we1a": np.asarray(inputs["w_e1"])[0:D, :],
        "we1b": np.asarray(inputs["w_e1"])[D:2 * D, :],
        "we2": inputs["w_e2"],
        "wsa": np.asarray(inputs["w_s"])[0:D, :],
        "wsb": np.asarray(inputs["w_s"])[D:2 * D, :],
    }
    sharedF = {
        "band": cfull["band"], "cm_band": cmini["band"],
        "be1": np.asarray(inputs["b_e1"], np.float32).reshape(D, 1),
        "be2": np.asarray(inputs["b_e2"], np.float32).reshape(NEMO, 1),
        "bs": np.asarray(inputs["b_s"], np.float32).reshape(NEMO, 1),
    }

    in_maps = []
    for r in range(CORES):
        lo = r * R
        xc = xg[lo:lo + XR]
        Ec = Eg[lo:lo + XR]
        eOz = np.zeros((EXT, NBL, NCLS), np.float32)
        for t in range(NBL):
            es = B + B * t
            eOz[WIN:WIN + B, t, 0:NSPK] = Ec[es + WIN:es + WIN + B]
            eOz[WIN:WIN + B, t, NSPK] = Ec[es + WIN:es + WIN + B].sum(axis=1)
        e9T = np.concatenate(
            [Ec[HALO:HALO + R].T,
             Ec[HALO:HALO + R].sum(axis=1)[None, :]], axis=0)
        Em = np.concatenate(
            [Ec[B:B + WIN], Ec[HALO + R:HALO + R + WIN]], axis=0)
        e9Tm = np.concatenate([Em.T, Em.sum(axis=1)[None, :]], axis=0)
        gblks = np.array([r * NBL + t for t in range(NBL)] +
                         [r * NBL - 1, (r + 1) * NBL])
        J = np.arange(NBG)[:, None]
        triS = (J > gblks[None, :]).astype(np.float32)
        triP = (J < gblks[None, :]).astype(np.float32)
        vm = np.ones((WIN, 2), np.float32)
        if r == 0:
            vm[:, 0] = 0.0
        if r == CORES - 1:
            vm[:, 1] = 0.0
        itemsS = {
            "eO": eOz, "e9Tm": e9Tm, "triS": triS, "triP": triP,
            "pred3": cfull["pred"], "suc3": cfull["suc"],
            "diagm3": cfull["diagm"],
            "predib": cfull["predib"], "sucib": cfull["sucib"],
            "cm_pred": cmini["pred"], "cm_suc": cmini["suc"],
            "cm_diagm": cmini["diagm"], "cm_predib": cmini["predib"],
            "cm_sucib": cmini["sucib"],
        }
        itemsF = dict(sharedF)
        itemsF.update({"vmask": vm})
        bf = ml_dtypes.bfloat16
        m = {
            "xT": np.ascontiguousarray(xc.T),
            "xTb": np.asarray(np.ascontiguousarray(xc.T), bf),
            "blobB": _pack_blob(sharedB, BLOB_B, COLS_B, bf, D),
            "blobS": _pack_blob(itemsS, BLOB_S, COLS_S, bf, EXT),
            "blobF": _pack_blob(itemsF, BLOB_F, COLS_F, np.float32, D),
            "eT": np.asarray(Ec.T, bf),
            "e9T": np.asarray(e9T, bf),
        }
        in_maps.append(m)
    return in_maps


_NC = None


def kernel(**inputs):
    global _NC
    if _NC is None:
        _NC = build_program()
    in_maps = make_in_maps(inputs)
    res = run_bass_kernel_spmd(_NC, in_maps, list(range(CORES)))
    emo = np.concatenate(
        [np.asarray(res.results[r]["emoT"]).T for r in range(CORES)], axis=0)
    sen = np.concatenate(
        [np.asarray(res.results[r]["senT"]).T for r in range(CORES)], axis=0)
    return emo, sen
